# revision 4
# baseline (speedup 1.0000x reference)
"""Chamfer distance (L1) Trainium2 Bass kernel.

Problem: xyz1 (4, 8192, 3) fp32, xyz2 (4, 8192, 3) fp32 ->
scalar = mean_b[ mean_n min_m ||x1-x2|| + mean_m min_n ||x1-x2|| ].

Strategy:
 - 8 cores: core c handles batch b=c//2, N-half h=c%2 -> a (4096 x 8192)
   distance block per core.
 - d2[n,m] = ||x1n||^2 + ||x2m||^2 - 2 x1n.x2m is computed as ONE matmul
   with an augmented contraction dim: K=16 rows of split-precision bf16
   (hi/lo decomposition) so bf16 PE speed with ~fp32 accuracy.
 - sqrt is monotone: min(sqrt(max(d2,0))) = sqrt(max(min(d2),0)), so all
   sqrt/mean work happens on host over only 12K values per core.
 - On device: row-direction min (over m) via DVE tensor_tensor(min) folds +
   one free-axis reduce per n-tile; col-direction min (over n) via elementwise
   min folds into a [128, 8192] accumulator, finished with PE transposes +
   free-axis reduces.
"""

import sys

sys.path.insert(0, "/opt/trn_rl_repo")

import numpy as np
import ml_dtypes

import concourse.bass as bass
import concourse.bacc as bacc
import concourse.mybir as mybir
import concourse.tile as tile
from concourse.bass_utils import run_bass_kernel_spmd

BF16 = mybir.dt.bfloat16
FP32 = mybir.dt.float32
NP_BF16 = ml_dtypes.bfloat16

B, N, M = 4, 8192, 8192
N_CORES = 8
NC_N = N // 2  # 4096 rows per core
K_AUG = 16

N_TILES = NC_N // 128  # 32
M_TILES_GROUP = M // 1024  # 16 groups of [128, 1024] psum tiles (2 matmuls each)


def build_program():
    nc = bacc.Bacc()

    lhs_d = nc.dram_tensor("lhs", [K_AUG, NC_N], BF16, kind="ExternalInput").ap()
    rhs_d = nc.dram_tensor("rhs", [K_AUG, M], BF16, kind="ExternalInput").ap()
    ident_d = nc.dram_tensor("ident", [128, 128], FP32, kind="ExternalInput").ap()
    rowmin_d = nc.dram_tensor(
        "rowmin", [128, N_TILES], FP32, kind="ExternalOutput"
    ).ap()
    colmin_d = nc.dram_tensor("colmin", [128, M // 128], FP32, kind="ExternalOutput").ap()

    amin = mybir.AluOpType.min
    ax_x = mybir.AxisListType.X

    with tile.TileContext(nc) as tc:
        with (
            tc.tile_pool(name="const", bufs=1) as const_pool,
            tc.tile_pool(name="acc", bufs=1) as acc_pool,
            tc.tile_pool(name="row", bufs=2) as row_pool,
            tc.tile_pool(name="out", bufs=1) as out_pool,
            tc.tile_pool(name="mm", bufs=3, space="PSUM") as mm_pool,
            tc.tile_pool(name="tr", bufs=2, space="PSUM") as tr_pool,
        ):
            lhs_sb = const_pool.tile([K_AUG, NC_N], BF16)
            rhs_sb = const_pool.tile([K_AUG, M], BF16)
            ident_sb = const_pool.tile([128, 128], FP32)
            nc.sync.dma_start(out=lhs_sb, in_=lhs_d)
            nc.sync.dma_start(out=rhs_sb, in_=rhs_d)
            nc.sync.dma_start(out=ident_sb, in_=ident_d)

            colacc = acc_pool.tile([128, M], FP32)  # fold over n-tiles
            rowmin_sb = out_pool.tile([128, N_TILES], FP32)
            colmin_sb = out_pool.tile([128, M // 128], FP32)

            for i in range(N_TILES):
                lhs_i = lhs_sb[:, i * 128 : (i + 1) * 128]
                rowacc = row_pool.tile([128, 1024], FP32)
                for jg in range(M_TILES_GROUP):
                    psum_t = mm_pool.tile([128, 1024], FP32)
                    for half in range(2):
                        j = jg * 2 + half
                        nc.tensor.matmul(
                            psum_t[:, half * 512 : (half + 1) * 512],
                            lhs_i,
                            rhs_sb[:, j * 512 : (j + 1) * 512],
                        )
                    # row-direction fold (over m)
                    if jg == 0:
                        nc.vector.tensor_copy(rowacc, psum_t)
                    else:
                        nc.vector.tensor_tensor(rowacc, rowacc, psum_t, amin)
                    # col-direction fold (over n)
                    cslice = colacc[:, jg * 1024 : (jg + 1) * 1024]
                    if i == 0:
                        nc.vector.tensor_copy(cslice, psum_t)
                    else:
                        nc.vector.tensor_tensor(cslice, cslice, psum_t, amin)
                nc.vector.tensor_reduce(
                    rowmin_sb[:, i : i + 1], rowacc, axis=ax_x, op=amin
                )

            # finish col-direction: transpose 128-wide chunks, then free-axis min
            for cc in range(M // 128):
                tr_t = tr_pool.tile([128, 128], FP32)
                nc.tensor.transpose(
                    tr_t, colacc[:, cc * 128 : (cc + 1) * 128], ident_sb
                )
                nc.vector.tensor_reduce(
                    colmin_sb[:, cc : cc + 1], tr_t, axis=ax_x, op=amin
                )

            nc.sync.dma_start(out=rowmin_d, in_=rowmin_sb)
            nc.sync.dma_start(out=colmin_d, in_=colmin_sb)

    nc.compile()
    return nc


def _split_bf16(v):
    """v (f64/f32 array) -> (hi, lo) bf16 with hi + lo ~= v."""
    hi = v.astype(NP_BF16)
    lo = (v.astype(np.float64) - hi.astype(np.float64)).astype(NP_BF16)
    return hi, lo


def _make_core_inputs(x1h, x2):
    """x1h (4096,3) fp32 slice, x2 (8192,3) fp32 -> lhs [16,4096], rhs [16,8192] bf16."""
    x1h = x1h.astype(np.float64)
    x2 = x2.astype(np.float64)
    h1, l1 = _split_bf16(x1h)
    h2, l2 = _split_bf16(x2)
    sq1 = (x1h * x1h).sum(-1)
    sq2 = (x2 * x2).sum(-1)
    s1h, s1l = _split_bf16(sq1)
    s2h, s2l = _split_bf16(sq2)

    n2h = (-2.0 * h2.astype(np.float64)).astype(NP_BF16)
    n2l = (-2.0 * l2.astype(np.float64)).astype(NP_BF16)

    ones_n = np.ones(NC_N, NP_BF16)
    ones_m = np.ones(M, NP_BF16)

    lhs = np.stack(
        [h1[:, 0], h1[:, 1], h1[:, 2]]
        + [h1[:, 0], h1[:, 1], h1[:, 2]]
        + [l1[:, 0], l1[:, 1], l1[:, 2]]
        + [l1[:, 0], l1[:, 1], l1[:, 2]]
        + [ones_n, ones_n, s1h, s1l]
    )
    rhs = np.stack(
        [n2h[:, 0], n2h[:, 1], n2h[:, 2]]
        + [n2l[:, 0], n2l[:, 1], n2l[:, 2]]
        + [n2h[:, 0], n2h[:, 1], n2h[:, 2]]
        + [n2l[:, 0], n2l[:, 1], n2l[:, 2]]
        + [s2h, s2l, ones_m, ones_m]
    )
    return np.ascontiguousarray(lhs), np.ascontiguousarray(rhs)


_CACHED_NC = None


def _get_nc():
    global _CACHED_NC
    if _CACHED_NC is None:
        _CACHED_NC = build_program()
    return _CACHED_NC


def kernel(xyz1, xyz2, _return_timing=False, _trace=False):
    xyz1 = np.asarray(xyz1, dtype=np.float32)
    xyz2 = np.asarray(xyz2, dtype=np.float32)
    assert xyz1.shape == (B, N, 3) and xyz2.shape == (B, M, 3)

    ident = np.eye(128, dtype=np.float32)
    in_maps = []
    for c in range(N_CORES):
        b, h = divmod(c, 2)
        lhs, rhs = _make_core_inputs(
            xyz1[b, h * NC_N : (h + 1) * NC_N], xyz2[b]
        )
        in_maps.append({"lhs": lhs, "rhs": rhs, "ident": ident})

    nc = _get_nc()
    res = run_bass_kernel_spmd(
        nc, in_maps, core_ids=list(range(N_CORES)), trace=_trace
    )

    total = 0.0
    for b in range(B):
        row_parts = []
        col_parts = []
        for h in range(2):
            r = res.results[2 * b + h]
            row_parts.append(np.asarray(r["rowmin"]).T.reshape(-1))  # (4096,)
            col_parts.append(np.asarray(r["colmin"]).T.reshape(-1))  # (8192,)
        min1_d2 = np.concatenate(row_parts)  # (8192,)
        min2_d2 = np.minimum(col_parts[0], col_parts[1])  # (8192,)
        min1 = np.sqrt(np.maximum(min1_d2, 0.0))
        min2 = np.sqrt(np.maximum(min2_d2, 0.0))
        total += min1.mean() + min2.mean()
    out = np.float32(total / B)
    if _return_timing:
        return out, res
    return out


# revision 8
# speedup vs baseline: 1.8498x; 1.8498x over previous
"""Chamfer distance (L1) Trainium2 Bass kernel.

Problem: xyz1 (4, 8192, 3) fp32, xyz2 (4, 8192, 3) fp32 ->
scalar = mean_b[ mean_n min_m ||x1-x2|| + mean_m min_n ||x1-x2|| ].

Strategy:
 - 8 cores: core c handles batch b=c//2, N-half h=c%2 -> a (4096 x 8192)
   distance block per core.
 - d2[n,m] = ||x1n||^2 + ||x2m||^2 - 2 x1n.x2m is computed as ONE matmul with
   an augmented contraction dim: K=33 rows of 3-level split-precision bf16
   (x = hi+mid+lo, all 9 cross products + 3-way split norms), giving ~fp32
   accuracy at bf16 PE speed (1 cyc/row vs 4 for fp32). Rows are ordered so
   PSUM partial sums stay small (cancellation early).
 - sqrt is monotone: min(sqrt(max(d2,0))) = sqrt(max(min(d2),0)), so sqrt and
   means happen on host over only 12K values per core.
 - ScalarE (ACT) drains each PSUM chunk to SBUF as fp16 scaled by 2^14 (free
   scale on the activation path; scaling keeps tiny d2 out of fp16
   subnormals, and overflow->inf is harmless under min).
 - VectorE does both min directions as fp16 tensor_tensor(min) folds in 2x
   mode: row-direction (over m) into rowacc + small reduce per n-tile;
   col-direction (over n) into a [128, 8192] accumulator, finished with PE
   transposes + free-axis reduces.
"""

import sys

sys.path.insert(0, "/opt/trn_rl_repo")

import numpy as np
import ml_dtypes

import concourse.bass as bass
import concourse.bacc as bacc
import concourse.mybir as mybir
import concourse.tile as tile
from concourse.bass_utils import run_bass_kernel_spmd

BF16 = mybir.dt.bfloat16
FP16 = mybir.dt.float16
FP32 = mybir.dt.float32
NP_BF16 = ml_dtypes.bfloat16

B, N, M = 4, 8192, 8192
N_CORES = 8
NC_N = N // 2  # 4096 rows per core
K_AUG = 33
D2_SCALE = 512.0  # 2^9: keeps d2*scale in fp16 normal range (max ~100*512 < 65504)

N_TILES = NC_N // 128  # 32
CHUNK = 2048  # psum chunk free size (4 matmuls of 512)
M_CHUNKS = M // CHUNK  # 4


def build_program():
    nc = bacc.Bacc()

    lhs_d = nc.dram_tensor("lhs", [K_AUG, NC_N], BF16, kind="ExternalInput").ap()
    rhs_d = nc.dram_tensor("rhs", [K_AUG, M], BF16, kind="ExternalInput").ap()
    ident_d = nc.dram_tensor("ident", [128, 128], FP16, kind="ExternalInput").ap()
    rowmin_d = nc.dram_tensor(
        "rowmin", [128, N_TILES], FP32, kind="ExternalOutput"
    ).ap()
    colmin_d = nc.dram_tensor(
        "colmin", [128, M // 128], FP32, kind="ExternalOutput"
    ).ap()

    amin = mybir.AluOpType.min
    ax_x = mybir.AxisListType.X

    with tile.TileContext(nc) as tc:
        with (
            tc.tile_pool(name="const", bufs=1) as const_pool,
            tc.tile_pool(name="acc", bufs=1) as acc_pool,
            tc.tile_pool(name="row", bufs=2) as row_pool,
            tc.tile_pool(name="drain", bufs=4) as drain_pool,
            tc.tile_pool(name="out", bufs=1) as out_pool,
            tc.tile_pool(name="mm", bufs=2, space="PSUM") as mm_pool,
        ):
            lhs_sb = const_pool.tile([K_AUG, NC_N], BF16)
            rhs_sb = const_pool.tile([K_AUG, M], BF16)
            ident_sb = const_pool.tile([128, 128], FP16)
            nc.sync.dma_start(out=lhs_sb, in_=lhs_d)
            nc.sync.dma_start(out=rhs_sb, in_=rhs_d)
            nc.sync.dma_start(out=ident_sb, in_=ident_d)

            colacc = acc_pool.tile([128, M], FP16)  # fold over n-tiles
            rowmin_sb = out_pool.tile([128, N_TILES], FP32)
            colmin_sb = out_pool.tile([128, M // 128], FP32)

            for i in range(N_TILES):
                lhs_i = lhs_sb[:, i * 128 : (i + 1) * 128]
                rowacc = row_pool.tile([128, CHUNK], FP16)
                for jg in range(M_CHUNKS):
                    psum_t = mm_pool.tile([128, CHUNK], FP32, tag="mm")
                    for q in range(CHUNK // 512):
                        j = jg * (CHUNK // 512) + q
                        nc.tensor.matmul(
                            psum_t[:, q * 512 : (q + 1) * 512],
                            lhs_i,
                            rhs_sb[:, j * 512 : (j + 1) * 512],
                        )
                    # ACT drains PSUM -> SBUF fp16 with free *2^14 scale
                    s_chunk = drain_pool.tile([128, CHUNK], FP16)
                    nc.scalar.mul(s_chunk, psum_t, D2_SCALE)
                    # row-direction fold (over m), fp16 2x mode
                    if jg == 0:
                        nc.vector.tensor_copy(rowacc, s_chunk)
                    else:
                        nc.vector.tensor_tensor(rowacc, rowacc, s_chunk, amin)
                    # col-direction fold (over n), fp16 2x mode
                    cslice = colacc[:, jg * CHUNK : (jg + 1) * CHUNK]
                    if i == 0:
                        nc.vector.tensor_copy(cslice, s_chunk)
                    else:
                        nc.vector.tensor_tensor(cslice, cslice, s_chunk, amin)
                # finish row-direction for this n-tile: halve twice, then reduce
                nc.vector.tensor_tensor(
                    rowacc[:, : CHUNK // 2],
                    rowacc[:, : CHUNK // 2],
                    rowacc[:, CHUNK // 2 :],
                    amin,
                )
                nc.vector.tensor_tensor(
                    rowacc[:, : CHUNK // 4],
                    rowacc[:, : CHUNK // 4],
                    rowacc[:, CHUNK // 4 : CHUNK // 2],
                    amin,
                )
                nc.vector.tensor_reduce(
                    rowmin_sb[:, i : i + 1],
                    rowacc[:, : CHUNK // 4],
                    axis=ax_x,
                    op=amin,
                )

            # clamp so a stray inf can't become NaN via the transpose matmul
            nc.vector.tensor_scalar_min(colacc, colacc, 60000.0)
            # finish col-direction: transpose 128-wide chunks, then free-axis min
            for cc in range(M // 128):
                tr_t = mm_pool.tile([128, 128], FP16, tag="mm")
                nc.tensor.transpose(
                    tr_t, colacc[:, cc * 128 : (cc + 1) * 128], ident_sb
                )
                nc.vector.tensor_reduce(
                    colmin_sb[:, cc : cc + 1], tr_t, axis=ax_x, op=amin
                )

            nc.sync.dma_start(out=rowmin_d, in_=rowmin_sb)
            nc.sync.dma_start(out=colmin_d, in_=colmin_sb)

    nc.compile()
    return nc


def _split3(v):
    """v (f64 array) -> (hi, mid, lo) bf16 with hi+mid+lo ~= v (~26-bit)."""
    v = v.astype(np.float64)
    hi = v.astype(NP_BF16)
    r1 = v - hi.astype(np.float64)
    mid = r1.astype(NP_BF16)
    lo = (r1 - mid.astype(np.float64)).astype(NP_BF16)
    return hi, mid, lo


def _make_core_inputs(x1h, x2):
    """x1h (4096,3), x2 (8192,3) fp32 -> lhs [33,4096], rhs [33,8192] bf16.

    Row pairing (lhs_k paired with rhs_k), ordered so PE partial sums cancel
    early: d2 = sq1 + sq2 - 2*x1.x2 with 3-level splits.
    """
    x1h = x1h.astype(np.float64)
    x2 = x2.astype(np.float64)
    a1 = _split3(x1h)  # (hi, mid, lo), each (4096, 3)
    a2 = _split3(x2)
    n2 = [(-2.0 * p.astype(np.float64)).astype(NP_BF16) for p in a2]  # exact *-2
    sq1 = (x1h * x1h).sum(-1)
    sq2 = (x2 * x2).sum(-1)
    s1 = _split3(sq1)
    s2 = _split3(sq2)

    ones_n = np.ones(NC_N, NP_BF16)
    ones_m = np.ones(M, NP_BF16)

    lhs_rows = []
    rhs_rows = []

    def add(l, r):
        lhs_rows.append(l)
        rhs_rows.append(r)

    # big terms first, interleaved for cancellation
    add(s1[0], ones_m)
    for d in range(3):
        add(a1[0][:, d], n2[0][:, d])  # hi*hi
    add(ones_n, s2[0])
    # mid-level terms
    add(s1[1], ones_m)
    add(ones_n, s2[1])
    for d in range(3):
        add(a1[0][:, d], n2[1][:, d])  # hi*mid
    for d in range(3):
        add(a1[1][:, d], n2[0][:, d])  # mid*hi
    for d in range(3):
        add(a1[1][:, d], n2[1][:, d])  # mid*mid
    # low-level terms
    add(s1[2], ones_m)
    add(ones_n, s2[2])
    for d in range(3):
        add(a1[0][:, d], n2[2][:, d])  # hi*lo
    for d in range(3):
        add(a1[2][:, d], n2[0][:, d])  # lo*hi
    for d in range(3):
        add(a1[1][:, d], n2[2][:, d])  # mid*lo
    for d in range(3):
        add(a1[2][:, d], n2[1][:, d])  # lo*mid
    for d in range(3):
        add(a1[2][:, d], n2[2][:, d])  # lo*lo

    lhs = np.ascontiguousarray(np.stack(lhs_rows))
    rhs = np.ascontiguousarray(np.stack(rhs_rows))
    assert lhs.shape == (K_AUG, NC_N) and rhs.shape == (K_AUG, M)
    return lhs, rhs


_CACHED_NC = None


def _get_nc():
    global _CACHED_NC
    if _CACHED_NC is None:
        _CACHED_NC = build_program()
    return _CACHED_NC


def kernel(xyz1, xyz2, _return_timing=False, _trace=False):
    xyz1 = np.asarray(xyz1, dtype=np.float32)
    xyz2 = np.asarray(xyz2, dtype=np.float32)
    assert xyz1.shape == (B, N, 3) and xyz2.shape == (B, M, 3)

    ident = np.eye(128, dtype=np.float16)
    in_maps = []
    for c in range(N_CORES):
        b, h = divmod(c, 2)
        lhs, rhs = _make_core_inputs(xyz1[b, h * NC_N : (h + 1) * NC_N], xyz2[b])
        in_maps.append({"lhs": lhs, "rhs": rhs, "ident": ident})

    nc = _get_nc()
    res = run_bass_kernel_spmd(
        nc, in_maps, core_ids=list(range(N_CORES)), trace=_trace
    )

    total = 0.0
    for b in range(B):
        row_parts = []
        col_parts = []
        for h in range(2):
            r = res.results[2 * b + h]
            row_parts.append(
                np.asarray(r["rowmin"]).astype(np.float64).T.reshape(-1)
            )  # (4096,)
            col_parts.append(
                np.asarray(r["colmin"]).astype(np.float64).T.reshape(-1)
            )  # (8192,)
        min1_d2 = np.concatenate(row_parts) / D2_SCALE  # (8192,)
        min2_d2 = np.minimum(col_parts[0], col_parts[1]) / D2_SCALE  # (8192,)
        min1 = np.sqrt(np.maximum(min1_d2, 0.0))
        min2 = np.sqrt(np.maximum(min2_d2, 0.0))
        total += min1.mean() + min2.mean()
    out = np.float32(total / B)
    if _return_timing:
        return out, res
    return out


# revision 13
# speedup vs baseline: 1.9051x; 1.0299x over previous
"""Chamfer distance (L1) Trainium2 Bass kernel.

Problem: xyz1 (4, 8192, 3) fp32, xyz2 (4, 8192, 3) fp32 ->
scalar = mean_b[ mean_n min_m ||x1-x2|| + mean_m min_n ||x1-x2|| ].

Strategy:
 - 8 cores: core c handles batch b=c//2, N-half h=c%2 -> a (4096 x 8192)
   distance block per core.
 - d2[n,m] = ||x1n||^2 + ||x2m||^2 - 2 x1n.x2m is computed as ONE matmul with
   an augmented contraction dim: K=33 rows of 3-level split-precision bf16
   (x = hi+mid+lo, all 9 cross products + 3-way split norms), giving ~fp32
   accuracy at bf16 PE speed (1 cyc/row vs 4 for fp32). Rows are ordered so
   PSUM partial sums stay small (cancellation early).
 - sqrt is monotone: min(sqrt(max(d2,0))) = sqrt(max(min(d2),0)), so sqrt and
   means happen on host over only 12K values per core.
 - ScalarE (ACT) drains each PSUM chunk to SBUF as fp16 scaled by 2^14 (free
   scale on the activation path; scaling keeps tiny d2 out of fp16
   subnormals, and overflow->inf is harmless under min).
 - VectorE does both min directions as fp16 tensor_tensor(min) folds in 2x
   mode: row-direction (over m) into rowacc + small reduce per n-tile;
   col-direction (over n) into a [128, 8192] accumulator, finished with PE
   transposes + free-axis reduces.
"""

import sys

sys.path.insert(0, "/opt/trn_rl_repo")

import numpy as np
import ml_dtypes

import concourse.bass as bass
import concourse.bacc as bacc
import concourse.mybir as mybir
import concourse.tile as tile
from concourse.bass_utils import run_bass_kernel_spmd

BF16 = mybir.dt.bfloat16
FP16 = mybir.dt.float16
FP32 = mybir.dt.float32
NP_BF16 = ml_dtypes.bfloat16

B, N, M = 4, 8192, 8192
N_CORES = 8
NC_N = N // 2  # 4096 rows per core
K_AUG = 33
D2_SCALE = 512.0  # 2^9: keeps d2*scale in fp16 normal range (max ~100*512 < 65504)

N_TILES = NC_N // 128  # 32
CHUNK = 2048  # psum chunk free size (4 matmuls of 512)
M_CHUNKS = M // CHUNK  # 4


def build_program():
    nc = bacc.Bacc()

    lhs_d = nc.dram_tensor("lhs", [K_AUG, NC_N], BF16, kind="ExternalInput").ap()
    rhs_d = nc.dram_tensor("rhs", [K_AUG, M], BF16, kind="ExternalInput").ap()
    ident_d = nc.dram_tensor("ident", [128, 128], FP16, kind="ExternalInput").ap()
    rowmin_d = nc.dram_tensor(
        "rowmin", [128, N_TILES], FP32, kind="ExternalOutput"
    ).ap()
    colmin_d = nc.dram_tensor(
        "colmin", [128, M // 128], FP32, kind="ExternalOutput"
    ).ap()

    amin = mybir.AluOpType.min
    ax_x = mybir.AxisListType.X

    with tile.TileContext(nc) as tc:
        with (
            tc.tile_pool(name="const", bufs=1) as const_pool,
            tc.tile_pool(name="acc", bufs=1) as acc_pool,
            tc.tile_pool(name="row", bufs=2) as row_pool,
            tc.tile_pool(name="drain", bufs=6) as drain_pool,
            tc.tile_pool(name="out", bufs=1) as out_pool,
            tc.tile_pool(name="mm", bufs=2, space="PSUM") as mm_pool,
        ):
            lhs_sb = const_pool.tile([K_AUG, NC_N], BF16)
            rhs_sb = const_pool.tile([K_AUG, M], BF16)
            ident_sb = const_pool.tile([128, 128], FP16)
            nc.sync.dma_start(out=lhs_sb, in_=lhs_d)
            nc.sync.dma_start(out=rhs_sb, in_=rhs_d)
            nc.sync.dma_start(out=ident_sb, in_=ident_d)

            colacc = acc_pool.tile([128, M], FP16)  # fold over n-tiles
            rowmin_sb = out_pool.tile([128, N_TILES], FP32)
            colmin_sb = out_pool.tile([128, M // 128], FP32)

            for i in range(N_TILES):
                lhs_i = lhs_sb[:, i * 128 : (i + 1) * 128]
                rowacc = row_pool.tile([128, CHUNK], FP16)
                for jg in range(M_CHUNKS):
                    psum_t = mm_pool.tile([128, CHUNK], FP32, tag="mm")
                    for q in range(CHUNK // 512):
                        j = jg * (CHUNK // 512) + q
                        nc.tensor.matmul(
                            psum_t[:, q * 512 : (q + 1) * 512],
                            lhs_i,
                            rhs_sb[:, j * 512 : (j + 1) * 512],
                        )
                    # ACT drains PSUM -> SBUF fp16 with free *2^14 scale
                    s_chunk = drain_pool.tile([128, CHUNK], FP16)
                    nc.scalar.mul(s_chunk, psum_t, D2_SCALE)
                    # row-direction fold (over m), fp16 2x mode
                    if jg == 0:
                        nc.vector.tensor_copy(rowacc, s_chunk)
                    else:
                        nc.vector.tensor_tensor(rowacc, rowacc, s_chunk, amin)
                    # col-direction fold (over n), fp16 2x mode
                    cslice = colacc[:, jg * CHUNK : (jg + 1) * CHUNK]
                    if i == 0:
                        nc.vector.tensor_copy(cslice, s_chunk)
                    else:
                        nc.vector.tensor_tensor(cslice, cslice, s_chunk, amin)
                # finish row-direction for this n-tile: halve twice, then reduce
                nc.vector.tensor_tensor(
                    rowacc[:, : CHUNK // 2],
                    rowacc[:, : CHUNK // 2],
                    rowacc[:, CHUNK // 2 :],
                    amin,
                )
                nc.vector.tensor_tensor(
                    rowacc[:, : CHUNK // 4],
                    rowacc[:, : CHUNK // 4],
                    rowacc[:, CHUNK // 4 : CHUNK // 2],
                    amin,
                )
                nc.vector.tensor_reduce(
                    rowmin_sb[:, i : i + 1],
                    rowacc[:, : CHUNK // 4],
                    axis=ax_x,
                    op=amin,
                )

            # clamp so a stray inf can't become NaN via the transpose matmul
            nc.vector.tensor_scalar_min(colacc, colacc, 60000.0)
            # finish col-direction: transpose 128-wide chunks (4 per PSUM tile),
            # then one fused free-axis min per group of 4
            for g in range(M // 512):
                tr_t = mm_pool.tile([128, 512], FP16, tag="mm")
                for c4 in range(4):
                    cc = g * 4 + c4
                    nc.tensor.transpose(
                        tr_t[:, c4 * 128 : (c4 + 1) * 128],
                        colacc[:, cc * 128 : (cc + 1) * 128],
                        ident_sb,
                    )
                nc.vector.tensor_reduce(
                    colmin_sb[:, g * 4 : (g + 1) * 4],
                    tr_t.rearrange("p (a b) -> p a b", b=128),
                    axis=ax_x,
                    op=amin,
                )

            nc.sync.dma_start(out=rowmin_d, in_=rowmin_sb)
            nc.sync.dma_start(out=colmin_d, in_=colmin_sb)

    nc.compile()
    return nc


def _split3(v):
    """v (f64 array) -> (hi, mid, lo) bf16 with hi+mid+lo ~= v (~26-bit)."""
    v = v.astype(np.float64)
    hi = v.astype(NP_BF16)
    r1 = v - hi.astype(np.float64)
    mid = r1.astype(NP_BF16)
    lo = (r1 - mid.astype(np.float64)).astype(NP_BF16)
    return hi, mid, lo


def _make_core_inputs(x1h, x2):
    """x1h (4096,3), x2 (8192,3) fp32 -> lhs [33,4096], rhs [33,8192] bf16.

    Row pairing (lhs_k paired with rhs_k), ordered so PE partial sums cancel
    early: d2 = sq1 + sq2 - 2*x1.x2 with 3-level splits.
    """
    x1h = x1h.astype(np.float64)
    x2 = x2.astype(np.float64)
    a1 = _split3(x1h)  # (hi, mid, lo), each (4096, 3)
    a2 = _split3(x2)
    n2 = [(-2.0 * p.astype(np.float64)).astype(NP_BF16) for p in a2]  # exact *-2
    sq1 = (x1h * x1h).sum(-1)
    sq2 = (x2 * x2).sum(-1)
    s1 = _split3(sq1)
    s2 = _split3(sq2)

    ones_n = np.ones(NC_N, NP_BF16)
    ones_m = np.ones(M, NP_BF16)

    lhs_rows = []
    rhs_rows = []

    def add(l, r):
        lhs_rows.append(l)
        rhs_rows.append(r)

    # big terms first, interleaved for cancellation
    add(s1[0], ones_m)
    for d in range(3):
        add(a1[0][:, d], n2[0][:, d])  # hi*hi
    add(ones_n, s2[0])
    # mid-level terms
    add(s1[1], ones_m)
    add(ones_n, s2[1])
    for d in range(3):
        add(a1[0][:, d], n2[1][:, d])  # hi*mid
    for d in range(3):
        add(a1[1][:, d], n2[0][:, d])  # mid*hi
    for d in range(3):
        add(a1[1][:, d], n2[1][:, d])  # mid*mid
    # low-level terms
    add(s1[2], ones_m)
    add(ones_n, s2[2])
    for d in range(3):
        add(a1[0][:, d], n2[2][:, d])  # hi*lo
    for d in range(3):
        add(a1[2][:, d], n2[0][:, d])  # lo*hi
    for d in range(3):
        add(a1[1][:, d], n2[2][:, d])  # mid*lo
    for d in range(3):
        add(a1[2][:, d], n2[1][:, d])  # lo*mid
    for d in range(3):
        add(a1[2][:, d], n2[2][:, d])  # lo*lo

    lhs = np.ascontiguousarray(np.stack(lhs_rows))
    rhs = np.ascontiguousarray(np.stack(rhs_rows))
    assert lhs.shape == (K_AUG, NC_N) and rhs.shape == (K_AUG, M)
    return lhs, rhs


_CACHED_NC = None


def _get_nc():
    global _CACHED_NC
    if _CACHED_NC is None:
        _CACHED_NC = build_program()
    return _CACHED_NC


def kernel(xyz1, xyz2, _return_timing=False, _trace=False):
    xyz1 = np.asarray(xyz1, dtype=np.float32)
    xyz2 = np.asarray(xyz2, dtype=np.float32)
    assert xyz1.shape == (B, N, 3) and xyz2.shape == (B, M, 3)

    ident = np.eye(128, dtype=np.float16)
    in_maps = []
    for c in range(N_CORES):
        b, h = divmod(c, 2)
        lhs, rhs = _make_core_inputs(xyz1[b, h * NC_N : (h + 1) * NC_N], xyz2[b])
        in_maps.append({"lhs": lhs, "rhs": rhs, "ident": ident})

    nc = _get_nc()
    res = run_bass_kernel_spmd(
        nc, in_maps, core_ids=list(range(N_CORES)), trace=_trace
    )

    total = 0.0
    for b in range(B):
        row_parts = []
        col_parts = []
        for h in range(2):
            r = res.results[2 * b + h]
            row_parts.append(
                np.asarray(r["rowmin"]).astype(np.float64).T.reshape(-1)
            )  # (4096,)
            col_parts.append(
                np.asarray(r["colmin"]).astype(np.float64).T.reshape(-1)
            )  # (8192,)
        min1_d2 = np.concatenate(row_parts) / D2_SCALE  # (8192,)
        min2_d2 = np.minimum(col_parts[0], col_parts[1]) / D2_SCALE  # (8192,)
        min1 = np.sqrt(np.maximum(min1_d2, 0.0))
        min2 = np.sqrt(np.maximum(min2_d2, 0.0))
        total += min1.mean() + min2.mean()
    out = np.asarray(total / B, dtype=np.float32)
    if _return_timing:
        return out, res
    return out


# revision 14
# speedup vs baseline: 2.0182x; 1.0594x over previous
"""Chamfer distance (L1) Trainium2 Bass kernel.

Problem: xyz1 (4, 8192, 3) fp32, xyz2 (4, 8192, 3) fp32 ->
scalar = mean_b[ mean_n min_m ||x1-x2|| + mean_m min_n ||x1-x2|| ].

Strategy:
 - 8 cores: core c handles batch b=c//2, N-half h=c%2 -> a (4096 x 8192)
   distance block per core.
 - d2[n,m] = ||x1n||^2 + ||x2m||^2 - 2 x1n.x2m is computed as ONE matmul with
   an augmented contraction dim: K=33 rows of 3-level split-precision bf16
   (x = hi+mid+lo, all 9 cross products + 3-way split norms), giving ~fp32
   accuracy at bf16 PE speed (1 cyc/row vs 4 for fp32). Rows are ordered so
   PSUM partial sums stay small (cancellation early).
 - sqrt is monotone: min(sqrt(max(d2,0))) = sqrt(max(min(d2),0)), so sqrt and
   means happen on host over only 12K values per core.
 - ScalarE (ACT) drains each PSUM chunk to SBUF as fp16 scaled by 2^14 (free
   scale on the activation path; scaling keeps tiny d2 out of fp16
   subnormals, and overflow->inf is harmless under min).
 - VectorE does both min directions as fp16 tensor_tensor(min) folds in 2x
   mode: row-direction (over m) into rowacc + small reduce per n-tile;
   col-direction (over n) into a [128, 8192] accumulator, finished with PE
   transposes + free-axis reduces.
"""

import sys

sys.path.insert(0, "/opt/trn_rl_repo")

import numpy as np
import ml_dtypes

import concourse.bass as bass
import concourse.bacc as bacc
import concourse.mybir as mybir
import concourse.tile as tile
from concourse.bass_utils import run_bass_kernel_spmd

BF16 = mybir.dt.bfloat16
FP16 = mybir.dt.float16
FP32 = mybir.dt.float32
NP_BF16 = ml_dtypes.bfloat16

B, N, M = 4, 8192, 8192
N_CORES = 8
NC_N = N // 2  # 4096 rows per core
K_AUG = 33
D2_SCALE = 512.0  # 2^9: keeps d2*scale in fp16 normal range (max ~100*512 < 65504)

N_TILES = NC_N // 128  # 32
CHUNK = 2048  # psum chunk free size (4 matmuls of 512)
M_CHUNKS = M // CHUNK  # 4


def build_program():
    nc = bacc.Bacc()

    lhs_d = nc.dram_tensor("lhs", [K_AUG, NC_N], BF16, kind="ExternalInput").ap()
    rhs_d = nc.dram_tensor("rhs", [K_AUG, M], BF16, kind="ExternalInput").ap()
    ident_d = nc.dram_tensor("ident", [128, 128], FP16, kind="ExternalInput").ap()
    rowmin_d = nc.dram_tensor(
        "rowmin", [128, N_TILES], FP32, kind="ExternalOutput"
    ).ap()
    colmin_d = nc.dram_tensor(
        "colmin", [128, M // 128], FP32, kind="ExternalOutput"
    ).ap()

    amin = mybir.AluOpType.min
    ax_x = mybir.AxisListType.X

    with tile.TileContext(nc) as tc:
        with (
            tc.tile_pool(name="const", bufs=1) as const_pool,
            tc.tile_pool(name="acc", bufs=1) as acc_pool,
            tc.tile_pool(name="row", bufs=2) as row_pool,
            tc.tile_pool(name="drain", bufs=6) as drain_pool,
            tc.tile_pool(name="out", bufs=1) as out_pool,
            tc.tile_pool(name="mm", bufs=2, space="PSUM") as mm_pool,
        ):
            lhs_sb = const_pool.tile([K_AUG, NC_N], BF16)
            rhs_sb = const_pool.tile([K_AUG, M], BF16)
            ident_sb = const_pool.tile([128, 128], FP16)
            nc.sync.dma_start(out=lhs_sb, in_=lhs_d)
            nc.sync.dma_start(out=rhs_sb, in_=rhs_d)
            nc.sync.dma_start(out=ident_sb, in_=ident_d)

            colacc = acc_pool.tile([128, M], FP16)  # fold over n-tiles
            rowmin_sb = out_pool.tile([128, N_TILES], FP32)
            colmin_sb = out_pool.tile([128, M // 128], FP32)

            for i in range(N_TILES):
                lhs_i = lhs_sb[:, i * 128 : (i + 1) * 128]
                rowacc = row_pool.tile([128, CHUNK], FP16)
                s_first = None
                for jg in range(M_CHUNKS):
                    psum_t = mm_pool.tile([128, CHUNK], FP32, tag="mm")
                    for q in range(CHUNK // 512):
                        j = jg * (CHUNK // 512) + q
                        nc.tensor.matmul(
                            psum_t[:, q * 512 : (q + 1) * 512],
                            lhs_i,
                            rhs_sb[:, j * 512 : (j + 1) * 512],
                        )
                    # ACT drains PSUM -> SBUF fp16 with free *D2_SCALE
                    s_chunk = drain_pool.tile([128, CHUNK], FP16)
                    nc.scalar.mul(s_chunk, psum_t, D2_SCALE)
                    # row-direction fold (over m), fp16 2x mode; first two
                    # chunks fold directly (saves an init copy)
                    if jg == 0:
                        s_first = s_chunk
                    elif jg == 1:
                        nc.vector.tensor_tensor(rowacc, s_first, s_chunk, amin)
                    else:
                        nc.vector.tensor_tensor(rowacc, rowacc, s_chunk, amin)
                    # col-direction fold (over n), fp16 2x mode
                    cslice = colacc[:, jg * CHUNK : (jg + 1) * CHUNK]
                    if i == 0:
                        nc.vector.tensor_copy(cslice, s_chunk)
                    else:
                        nc.vector.tensor_tensor(cslice, cslice, s_chunk, amin)
                # finish row-direction for this n-tile: halve 3x, then reduce
                nc.vector.tensor_tensor(
                    rowacc[:, : CHUNK // 2],
                    rowacc[:, : CHUNK // 2],
                    rowacc[:, CHUNK // 2 :],
                    amin,
                )
                nc.vector.tensor_tensor(
                    rowacc[:, : CHUNK // 4],
                    rowacc[:, : CHUNK // 4],
                    rowacc[:, CHUNK // 4 : CHUNK // 2],
                    amin,
                )
                nc.vector.tensor_tensor(
                    rowacc[:, : CHUNK // 8],
                    rowacc[:, : CHUNK // 8],
                    rowacc[:, CHUNK // 8 : CHUNK // 4],
                    amin,
                )
                nc.vector.tensor_reduce(
                    rowmin_sb[:, i : i + 1],
                    rowacc[:, : CHUNK // 8],
                    axis=ax_x,
                    op=amin,
                )

            # clamp so a stray inf can't become NaN via the transpose matmul
            nc.vector.tensor_scalar_min(colacc, colacc, 60000.0)
            # finish col-direction: transpose 128-wide chunks (4 per PSUM tile),
            # then one fused free-axis min per group of 4
            for g in range(M // 512):
                tr_t = mm_pool.tile([128, 512], FP16, tag="mm")
                for c4 in range(4):
                    cc = g * 4 + c4
                    nc.tensor.transpose(
                        tr_t[:, c4 * 128 : (c4 + 1) * 128],
                        colacc[:, cc * 128 : (cc + 1) * 128],
                        ident_sb,
                    )
                nc.vector.tensor_reduce(
                    colmin_sb[:, g * 4 : (g + 1) * 4],
                    tr_t.rearrange("p (a b) -> p a b", b=128),
                    axis=ax_x,
                    op=amin,
                )

            nc.sync.dma_start(out=rowmin_d, in_=rowmin_sb)
            nc.sync.dma_start(out=colmin_d, in_=colmin_sb)

    nc.compile()
    return nc


def _split3(v):
    """v (f64 array) -> (hi, mid, lo) bf16 with hi+mid+lo ~= v (~26-bit)."""
    v = v.astype(np.float64)
    hi = v.astype(NP_BF16)
    r1 = v - hi.astype(np.float64)
    mid = r1.astype(NP_BF16)
    lo = (r1 - mid.astype(np.float64)).astype(NP_BF16)
    return hi, mid, lo


def _make_core_inputs(x1h, x2):
    """x1h (4096,3), x2 (8192,3) fp32 -> lhs [33,4096], rhs [33,8192] bf16.

    Row pairing (lhs_k paired with rhs_k), ordered so PE partial sums cancel
    early: d2 = sq1 + sq2 - 2*x1.x2 with 3-level splits.
    """
    x1h = x1h.astype(np.float64)
    x2 = x2.astype(np.float64)
    a1 = _split3(x1h)  # (hi, mid, lo), each (4096, 3)
    a2 = _split3(x2)
    n2 = [(-2.0 * p.astype(np.float64)).astype(NP_BF16) for p in a2]  # exact *-2
    sq1 = (x1h * x1h).sum(-1)
    sq2 = (x2 * x2).sum(-1)
    s1 = _split3(sq1)
    s2 = _split3(sq2)

    ones_n = np.ones(NC_N, NP_BF16)
    ones_m = np.ones(M, NP_BF16)

    lhs_rows = []
    rhs_rows = []

    def add(l, r):
        lhs_rows.append(l)
        rhs_rows.append(r)

    # big terms first, interleaved for cancellation
    add(s1[0], ones_m)
    for d in range(3):
        add(a1[0][:, d], n2[0][:, d])  # hi*hi
    add(ones_n, s2[0])
    # mid-level terms
    add(s1[1], ones_m)
    add(ones_n, s2[1])
    for d in range(3):
        add(a1[0][:, d], n2[1][:, d])  # hi*mid
    for d in range(3):
        add(a1[1][:, d], n2[0][:, d])  # mid*hi
    for d in range(3):
        add(a1[1][:, d], n2[1][:, d])  # mid*mid
    # low-level terms
    add(s1[2], ones_m)
    add(ones_n, s2[2])
    for d in range(3):
        add(a1[0][:, d], n2[2][:, d])  # hi*lo
    for d in range(3):
        add(a1[2][:, d], n2[0][:, d])  # lo*hi
    for d in range(3):
        add(a1[1][:, d], n2[2][:, d])  # mid*lo
    for d in range(3):
        add(a1[2][:, d], n2[1][:, d])  # lo*mid
    for d in range(3):
        add(a1[2][:, d], n2[2][:, d])  # lo*lo

    lhs = np.ascontiguousarray(np.stack(lhs_rows))
    rhs = np.ascontiguousarray(np.stack(rhs_rows))
    assert lhs.shape == (K_AUG, NC_N) and rhs.shape == (K_AUG, M)
    return lhs, rhs


_CACHED_NC = None


def _get_nc():
    global _CACHED_NC
    if _CACHED_NC is None:
        _CACHED_NC = build_program()
    return _CACHED_NC


def kernel(xyz1, xyz2, _return_timing=False, _trace=False):
    xyz1 = np.asarray(xyz1, dtype=np.float32)
    xyz2 = np.asarray(xyz2, dtype=np.float32)
    assert xyz1.shape == (B, N, 3) and xyz2.shape == (B, M, 3)

    ident = np.eye(128, dtype=np.float16)
    in_maps = []
    for c in range(N_CORES):
        b, h = divmod(c, 2)
        lhs, rhs = _make_core_inputs(xyz1[b, h * NC_N : (h + 1) * NC_N], xyz2[b])
        in_maps.append({"lhs": lhs, "rhs": rhs, "ident": ident})

    nc = _get_nc()
    res = run_bass_kernel_spmd(
        nc, in_maps, core_ids=list(range(N_CORES)), trace=_trace
    )

    total = 0.0
    for b in range(B):
        row_parts = []
        col_parts = []
        for h in range(2):
            r = res.results[2 * b + h]
            row_parts.append(
                np.asarray(r["rowmin"]).astype(np.float64).T.reshape(-1)
            )  # (4096,)
            col_parts.append(
                np.asarray(r["colmin"]).astype(np.float64).T.reshape(-1)
            )  # (8192,)
        min1_d2 = np.concatenate(row_parts) / D2_SCALE  # (8192,)
        min2_d2 = np.minimum(col_parts[0], col_parts[1]) / D2_SCALE  # (8192,)
        min1 = np.sqrt(np.maximum(min1_d2, 0.0))
        min2 = np.sqrt(np.maximum(min2_d2, 0.0))
        total += min1.mean() + min2.mean()
    out = np.asarray(total / B, dtype=np.float32)
    if _return_timing:
        return out, res
    return out


# revision 15
# speedup vs baseline: 2.0187x; 1.0003x over previous
"""Chamfer distance (L1) Trainium2 Bass kernel.

Problem: xyz1 (4, 8192, 3) fp32, xyz2 (4, 8192, 3) fp32 ->
scalar = mean_b[ mean_n min_m ||x1-x2|| + mean_m min_n ||x1-x2|| ].

Strategy:
 - 8 cores: core c handles batch b=c//2, N-half h=c%2 -> a (4096 x 8192)
   distance block per core.
 - d2[n,m] = ||x1n||^2 + ||x2m||^2 - 2 x1n.x2m is computed as ONE matmul with
   an augmented contraction dim: K=33 rows of 3-level split-precision bf16
   (x = hi+mid+lo, all 9 cross products + 3-way split norms), giving ~fp32
   accuracy at bf16 PE speed (1 cyc/row vs 4 for fp32). Rows are ordered so
   PSUM partial sums stay small (cancellation early).
 - sqrt is monotone: min(sqrt(max(d2,0))) = sqrt(max(min(d2),0)), so sqrt and
   means happen on host over only 12K values per core.
 - ScalarE (ACT) drains each PSUM chunk to SBUF as fp16 scaled by 2^14 (free
   scale on the activation path; scaling keeps tiny d2 out of fp16
   subnormals, and overflow->inf is harmless under min).
 - VectorE does both min directions as fp16 tensor_tensor(min) folds in 2x
   mode: row-direction (over m) into rowacc + small reduce per n-tile;
   col-direction (over n) into a [128, 8192] accumulator, finished with PE
   transposes + free-axis reduces.
"""

import sys

sys.path.insert(0, "/opt/trn_rl_repo")

import numpy as np
import ml_dtypes

import concourse.bass as bass
import concourse.bacc as bacc
import concourse.mybir as mybir
import concourse.tile as tile
from concourse.bass_utils import run_bass_kernel_spmd

BF16 = mybir.dt.bfloat16
FP16 = mybir.dt.float16
FP32 = mybir.dt.float32
NP_BF16 = ml_dtypes.bfloat16

B, N, M = 4, 8192, 8192
N_CORES = 8
NC_N = N // 2  # 4096 rows per core
K_AUG = 33
D2_SCALE = 512.0  # 2^9: keeps d2*scale in fp16 normal range (max ~100*512 < 65504)

N_TILES = NC_N // 128  # 32
CHUNK = 2048  # psum chunk free size (4 matmuls of 512)
M_CHUNKS = M // CHUNK  # 4


def build_program():
    nc = bacc.Bacc()

    lhs_d = nc.dram_tensor("lhs", [K_AUG, NC_N], BF16, kind="ExternalInput").ap()
    rhs_d = nc.dram_tensor("rhs", [K_AUG, M], BF16, kind="ExternalInput").ap()
    ident_d = nc.dram_tensor("ident", [128, 128], FP16, kind="ExternalInput").ap()
    rowmin_d = nc.dram_tensor(
        "rowmin", [128, N_TILES], FP32, kind="ExternalOutput"
    ).ap()
    colmin_d = nc.dram_tensor(
        "colmin", [128, M // 128], FP32, kind="ExternalOutput"
    ).ap()

    amin = mybir.AluOpType.min
    ax_x = mybir.AxisListType.X

    with tile.TileContext(nc) as tc:
        with (
            tc.tile_pool(name="const", bufs=1) as const_pool,
            tc.tile_pool(name="acc", bufs=1) as acc_pool,
            tc.tile_pool(name="row", bufs=3) as row_pool,
            tc.tile_pool(name="drain", bufs=8) as drain_pool,
            tc.tile_pool(name="out", bufs=1) as out_pool,
            tc.tile_pool(name="mm", bufs=2, space="PSUM") as mm_pool,
        ):
            lhs_sb = const_pool.tile([K_AUG, NC_N], BF16)
            rhs_sb = const_pool.tile([K_AUG, M], BF16)
            ident_sb = const_pool.tile([128, 128], FP16)
            nc.sync.dma_start(out=lhs_sb, in_=lhs_d)
            nc.sync.dma_start(out=rhs_sb, in_=rhs_d)
            nc.sync.dma_start(out=ident_sb, in_=ident_d)

            colacc = acc_pool.tile([128, M], FP16)  # fold over n-tiles
            rowmin_sb = out_pool.tile([128, N_TILES], FP32)
            colmin_sb = out_pool.tile([128, M // 128], FP32)

            for i in range(N_TILES):
                lhs_i = lhs_sb[:, i * 128 : (i + 1) * 128]
                rowacc = row_pool.tile([128, CHUNK], FP16)
                s_first = None
                for jg in range(M_CHUNKS):
                    psum_t = mm_pool.tile([128, CHUNK], FP32, tag="mm")
                    for q in range(CHUNK // 512):
                        j = jg * (CHUNK // 512) + q
                        nc.tensor.matmul(
                            psum_t[:, q * 512 : (q + 1) * 512],
                            lhs_i,
                            rhs_sb[:, j * 512 : (j + 1) * 512],
                        )
                    # ACT drains PSUM -> SBUF fp16 with free *D2_SCALE
                    s_chunk = drain_pool.tile([128, CHUNK], FP16)
                    nc.scalar.mul(s_chunk, psum_t, D2_SCALE)
                    # row-direction fold (over m), fp16 2x mode; first two
                    # chunks fold directly (saves an init copy)
                    if jg == 0:
                        s_first = s_chunk
                    elif jg == 1:
                        nc.vector.tensor_tensor(rowacc, s_first, s_chunk, amin)
                    else:
                        nc.vector.tensor_tensor(rowacc, rowacc, s_chunk, amin)
                    # col-direction fold (over n), fp16 2x mode
                    cslice = colacc[:, jg * CHUNK : (jg + 1) * CHUNK]
                    if i == 0:
                        nc.vector.tensor_copy(cslice, s_chunk)
                    else:
                        nc.vector.tensor_tensor(cslice, cslice, s_chunk, amin)
                # finish row-direction for this n-tile: halve 3x, then reduce
                nc.vector.tensor_tensor(
                    rowacc[:, : CHUNK // 2],
                    rowacc[:, : CHUNK // 2],
                    rowacc[:, CHUNK // 2 :],
                    amin,
                )
                nc.vector.tensor_tensor(
                    rowacc[:, : CHUNK // 4],
                    rowacc[:, : CHUNK // 4],
                    rowacc[:, CHUNK // 4 : CHUNK // 2],
                    amin,
                )
                nc.vector.tensor_tensor(
                    rowacc[:, : CHUNK // 8],
                    rowacc[:, : CHUNK // 8],
                    rowacc[:, CHUNK // 8 : CHUNK // 4],
                    amin,
                )
                nc.vector.tensor_reduce(
                    rowmin_sb[:, i : i + 1],
                    rowacc[:, : CHUNK // 8],
                    axis=ax_x,
                    op=amin,
                )

            # clamp so a stray inf can't become NaN via the transpose matmul
            nc.vector.tensor_scalar_min(colacc, colacc, 60000.0)
            # finish col-direction: transpose 128-wide chunks (4 per PSUM tile),
            # then one fused free-axis min per group of 4
            for g in range(M // 512):
                tr_t = mm_pool.tile([128, 512], FP16, tag="mm")
                for c4 in range(4):
                    cc = g * 4 + c4
                    nc.tensor.transpose(
                        tr_t[:, c4 * 128 : (c4 + 1) * 128],
                        colacc[:, cc * 128 : (cc + 1) * 128],
                        ident_sb,
                    )
                nc.vector.tensor_reduce(
                    colmin_sb[:, g * 4 : (g + 1) * 4],
                    tr_t.rearrange("p (a b) -> p a b", b=128),
                    axis=ax_x,
                    op=amin,
                )

            nc.sync.dma_start(out=rowmin_d, in_=rowmin_sb)
            nc.sync.dma_start(out=colmin_d, in_=colmin_sb)

    nc.compile()
    return nc


def _split3(v):
    """v (f64 array) -> (hi, mid, lo) bf16 with hi+mid+lo ~= v (~26-bit)."""
    v = v.astype(np.float64)
    hi = v.astype(NP_BF16)
    r1 = v - hi.astype(np.float64)
    mid = r1.astype(NP_BF16)
    lo = (r1 - mid.astype(np.float64)).astype(NP_BF16)
    return hi, mid, lo


def _make_core_inputs(x1h, x2):
    """x1h (4096,3), x2 (8192,3) fp32 -> lhs [33,4096], rhs [33,8192] bf16.

    Row pairing (lhs_k paired with rhs_k), ordered so PE partial sums cancel
    early: d2 = sq1 + sq2 - 2*x1.x2 with 3-level splits.
    """
    x1h = x1h.astype(np.float64)
    x2 = x2.astype(np.float64)
    a1 = _split3(x1h)  # (hi, mid, lo), each (4096, 3)
    a2 = _split3(x2)
    n2 = [(-2.0 * p.astype(np.float64)).astype(NP_BF16) for p in a2]  # exact *-2
    sq1 = (x1h * x1h).sum(-1)
    sq2 = (x2 * x2).sum(-1)
    s1 = _split3(sq1)
    s2 = _split3(sq2)

    ones_n = np.ones(NC_N, NP_BF16)
    ones_m = np.ones(M, NP_BF16)

    lhs_rows = []
    rhs_rows = []

    def add(l, r):
        lhs_rows.append(l)
        rhs_rows.append(r)

    # big terms first, interleaved for cancellation
    add(s1[0], ones_m)
    for d in range(3):
        add(a1[0][:, d], n2[0][:, d])  # hi*hi
    add(ones_n, s2[0])
    # mid-level terms
    add(s1[1], ones_m)
    add(ones_n, s2[1])
    for d in range(3):
        add(a1[0][:, d], n2[1][:, d])  # hi*mid
    for d in range(3):
        add(a1[1][:, d], n2[0][:, d])  # mid*hi
    for d in range(3):
        add(a1[1][:, d], n2[1][:, d])  # mid*mid
    # low-level terms
    add(s1[2], ones_m)
    add(ones_n, s2[2])
    for d in range(3):
        add(a1[0][:, d], n2[2][:, d])  # hi*lo
    for d in range(3):
        add(a1[2][:, d], n2[0][:, d])  # lo*hi
    for d in range(3):
        add(a1[1][:, d], n2[2][:, d])  # mid*lo
    for d in range(3):
        add(a1[2][:, d], n2[1][:, d])  # lo*mid
    for d in range(3):
        add(a1[2][:, d], n2[2][:, d])  # lo*lo

    lhs = np.ascontiguousarray(np.stack(lhs_rows))
    rhs = np.ascontiguousarray(np.stack(rhs_rows))
    assert lhs.shape == (K_AUG, NC_N) and rhs.shape == (K_AUG, M)
    return lhs, rhs


_CACHED_NC = None


def _get_nc():
    global _CACHED_NC
    if _CACHED_NC is None:
        _CACHED_NC = build_program()
    return _CACHED_NC


def kernel(xyz1, xyz2, _return_timing=False, _trace=False):
    xyz1 = np.asarray(xyz1, dtype=np.float32)
    xyz2 = np.asarray(xyz2, dtype=np.float32)
    assert xyz1.shape == (B, N, 3) and xyz2.shape == (B, M, 3)

    ident = np.eye(128, dtype=np.float16)
    in_maps = []
    for c in range(N_CORES):
        b, h = divmod(c, 2)
        lhs, rhs = _make_core_inputs(xyz1[b, h * NC_N : (h + 1) * NC_N], xyz2[b])
        in_maps.append({"lhs": lhs, "rhs": rhs, "ident": ident})

    nc = _get_nc()
    res = run_bass_kernel_spmd(
        nc, in_maps, core_ids=list(range(N_CORES)), trace=_trace
    )

    total = 0.0
    for b in range(B):
        row_parts = []
        col_parts = []
        for h in range(2):
            r = res.results[2 * b + h]
            row_parts.append(
                np.asarray(r["rowmin"]).astype(np.float64).T.reshape(-1)
            )  # (4096,)
            col_parts.append(
                np.asarray(r["colmin"]).astype(np.float64).T.reshape(-1)
            )  # (8192,)
        min1_d2 = np.concatenate(row_parts) / D2_SCALE  # (8192,)
        min2_d2 = np.minimum(col_parts[0], col_parts[1]) / D2_SCALE  # (8192,)
        min1 = np.sqrt(np.maximum(min1_d2, 0.0))
        min2 = np.sqrt(np.maximum(min2_d2, 0.0))
        total += min1.mean() + min2.mean()
    out = np.asarray(total / B, dtype=np.float32)
    if _return_timing:
        return out, res
    return out


# revision 17
# speedup vs baseline: 2.0383x; 1.0097x over previous
"""Chamfer distance (L1) Trainium2 Bass kernel.

Problem: xyz1 (4, 8192, 3) fp32, xyz2 (4, 8192, 3) fp32 ->
scalar = mean_b[ mean_n min_m ||x1-x2|| + mean_m min_n ||x1-x2|| ].

Strategy:
 - 8 cores: core c handles batch b=c//2, N-half h=c%2 -> a (4096 x 8192)
   distance block per core.
 - d2[n,m] = ||x1n||^2 + ||x2m||^2 - 2 x1n.x2m is computed as ONE matmul with
   an augmented contraction dim: K=33 rows of 3-level split-precision bf16
   (x = hi+mid+lo, all 9 cross products + 3-way split norms), giving ~fp32
   accuracy at bf16 PE speed (1 cyc/row vs 4 for fp32). Rows are ordered so
   PSUM partial sums stay small (cancellation early).
 - sqrt is monotone: min(sqrt(max(d2,0))) = sqrt(max(min(d2),0)), so sqrt and
   means happen on host over only 12K values per core.
 - ScalarE (ACT) drains each PSUM chunk to SBUF as fp16 scaled by 2^14 (free
   scale on the activation path; scaling keeps tiny d2 out of fp16
   subnormals, and overflow->inf is harmless under min).
 - VectorE does both min directions as fp16 tensor_tensor(min) folds in 2x
   mode: row-direction (over m) into rowacc + small reduce per n-tile;
   col-direction (over n) into a [128, 8192] accumulator, finished with PE
   transposes + free-axis reduces.
"""

import sys

sys.path.insert(0, "/opt/trn_rl_repo")

import numpy as np
import ml_dtypes

import concourse.bass as bass
import concourse.bacc as bacc
import concourse.mybir as mybir
import concourse.tile as tile
from concourse.bass_utils import run_bass_kernel_spmd

BF16 = mybir.dt.bfloat16
FP16 = mybir.dt.float16
FP32 = mybir.dt.float32
NP_BF16 = ml_dtypes.bfloat16

B, N, M = 4, 8192, 8192
N_CORES = 8
NC_N = N // 2  # 4096 rows per core
K_AUG = 33
D2_SCALE = 512.0  # 2^9: keeps d2*scale in fp16 normal range (max ~100*512 < 65504)

N_TILES = NC_N // 128  # 32
CHUNK = 2048  # psum chunk free size (4 matmuls of 512)
M_CHUNKS = M // CHUNK  # 4


def build_program():
    nc = bacc.Bacc()

    lhs_d = nc.dram_tensor("lhs", [K_AUG, NC_N], BF16, kind="ExternalInput").ap()
    rhs_d = nc.dram_tensor("rhs", [K_AUG, M], BF16, kind="ExternalInput").ap()
    ident_d = nc.dram_tensor("ident", [128, 128], FP16, kind="ExternalInput").ap()
    rowmin_d = nc.dram_tensor(
        "rowmin", [128, N_TILES], FP32, kind="ExternalOutput"
    ).ap()
    colmin_d = nc.dram_tensor(
        "colmin", [128, M // 128], FP32, kind="ExternalOutput"
    ).ap()

    amin = mybir.AluOpType.min
    ax_x = mybir.AxisListType.X

    with tile.TileContext(nc) as tc:
        with (
            tc.tile_pool(name="const", bufs=1) as const_pool,
            tc.tile_pool(name="acc", bufs=1) as acc_pool,
            tc.tile_pool(name="row", bufs=3) as row_pool,
            tc.tile_pool(name="drain", bufs=4) as drain_pool,
            tc.tile_pool(name="out", bufs=1) as out_pool,
            tc.tile_pool(name="mm", bufs=2, space="PSUM") as mm_pool,
        ):
            lhs_sb = const_pool.tile([K_AUG, NC_N], BF16)
            rhs_sb = const_pool.tile([K_AUG, M], BF16)
            ident_sb = const_pool.tile([128, 128], FP16)
            nc.sync.dma_start(out=lhs_sb, in_=lhs_d)
            nc.sync.dma_start(out=rhs_sb, in_=rhs_d)
            nc.sync.dma_start(out=ident_sb, in_=ident_d)

            colacc = acc_pool.tile([128, M], FP16)  # fold over n-tiles
            rowmin_sb = out_pool.tile([128, N_TILES], FP32)
            colmin_sb = out_pool.tile([128, M // 128], FP32)

            for i in range(N_TILES):
                lhs_i = lhs_sb[:, i * 128 : (i + 1) * 128]
                rowacc = row_pool.tile([128, CHUNK], FP16)
                for jp in range(M_CHUNKS // 2):
                    # drain a PAIR of psum chunks into one [128,4096] tile so
                    # the col-direction fold runs as one wide 4096 op
                    pair = drain_pool.tile([128, 2 * CHUNK], FP16)
                    for half in range(2):
                        jg = jp * 2 + half
                        psum_t = mm_pool.tile([128, CHUNK], FP32, tag="mm")
                        for q in range(CHUNK // 512):
                            j = jg * (CHUNK // 512) + q
                            nc.tensor.matmul(
                                psum_t[:, q * 512 : (q + 1) * 512],
                                lhs_i,
                                rhs_sb[:, j * 512 : (j + 1) * 512],
                            )
                        # ACT drains PSUM -> SBUF fp16 with free *D2_SCALE
                        nc.scalar.mul(
                            pair[:, half * CHUNK : (half + 1) * CHUNK],
                            psum_t,
                            D2_SCALE,
                        )
                    # row-direction fold (over m), fp16 2x mode
                    if jp == 0:
                        nc.vector.tensor_tensor(
                            rowacc, pair[:, :CHUNK], pair[:, CHUNK:], amin
                        )
                    else:
                        nc.vector.tensor_tensor(rowacc, rowacc, pair[:, :CHUNK], amin)
                        nc.vector.tensor_tensor(rowacc, rowacc, pair[:, CHUNK:], amin)
                    # col-direction fold (over n), one wide fp16 2x op
                    cslice = colacc[:, jp * 2 * CHUNK : (jp + 1) * 2 * CHUNK]
                    if i == 0:
                        nc.vector.tensor_copy(cslice, pair)
                    else:
                        nc.vector.tensor_tensor(cslice, cslice, pair, amin)
                # finish row-direction for this n-tile: halve 3x, then reduce
                nc.vector.tensor_tensor(
                    rowacc[:, : CHUNK // 2],
                    rowacc[:, : CHUNK // 2],
                    rowacc[:, CHUNK // 2 :],
                    amin,
                )
                nc.vector.tensor_tensor(
                    rowacc[:, : CHUNK // 4],
                    rowacc[:, : CHUNK // 4],
                    rowacc[:, CHUNK // 4 : CHUNK // 2],
                    amin,
                )
                nc.vector.tensor_tensor(
                    rowacc[:, : CHUNK // 8],
                    rowacc[:, : CHUNK // 8],
                    rowacc[:, CHUNK // 8 : CHUNK // 4],
                    amin,
                )
                nc.vector.tensor_reduce(
                    rowmin_sb[:, i : i + 1],
                    rowacc[:, : CHUNK // 8],
                    axis=ax_x,
                    op=amin,
                )

            # clamp so a stray inf can't become NaN via the transpose matmul
            nc.vector.tensor_scalar_min(colacc, colacc, 60000.0)
            # finish col-direction: transpose 128-wide chunks (4 per PSUM tile),
            # then one fused free-axis min per group of 4
            for g in range(M // 512):
                tr_t = mm_pool.tile([128, 512], FP16, tag="mm")
                for c4 in range(4):
                    cc = g * 4 + c4
                    nc.tensor.transpose(
                        tr_t[:, c4 * 128 : (c4 + 1) * 128],
                        colacc[:, cc * 128 : (cc + 1) * 128],
                        ident_sb,
                    )
                nc.vector.tensor_reduce(
                    colmin_sb[:, g * 4 : (g + 1) * 4],
                    tr_t.rearrange("p (a b) -> p a b", b=128),
                    axis=ax_x,
                    op=amin,
                )

            nc.sync.dma_start(out=rowmin_d, in_=rowmin_sb)
            nc.sync.dma_start(out=colmin_d, in_=colmin_sb)

    nc.compile()
    return nc


def _split3(v):
    """v (f64 array) -> (hi, mid, lo) bf16 with hi+mid+lo ~= v (~26-bit)."""
    v = v.astype(np.float64)
    hi = v.astype(NP_BF16)
    r1 = v - hi.astype(np.float64)
    mid = r1.astype(NP_BF16)
    lo = (r1 - mid.astype(np.float64)).astype(NP_BF16)
    return hi, mid, lo


def _make_core_inputs(x1h, x2):
    """x1h (4096,3), x2 (8192,3) fp32 -> lhs [33,4096], rhs [33,8192] bf16.

    Row pairing (lhs_k paired with rhs_k), ordered so PE partial sums cancel
    early: d2 = sq1 + sq2 - 2*x1.x2 with 3-level splits.
    """
    x1h = x1h.astype(np.float64)
    x2 = x2.astype(np.float64)
    a1 = _split3(x1h)  # (hi, mid, lo), each (4096, 3)
    a2 = _split3(x2)
    n2 = [(-2.0 * p.astype(np.float64)).astype(NP_BF16) for p in a2]  # exact *-2
    sq1 = (x1h * x1h).sum(-1)
    sq2 = (x2 * x2).sum(-1)
    s1 = _split3(sq1)
    s2 = _split3(sq2)

    ones_n = np.ones(NC_N, NP_BF16)
    ones_m = np.ones(M, NP_BF16)

    lhs_rows = []
    rhs_rows = []

    def add(l, r):
        lhs_rows.append(l)
        rhs_rows.append(r)

    # big terms first, interleaved for cancellation
    add(s1[0], ones_m)
    for d in range(3):
        add(a1[0][:, d], n2[0][:, d])  # hi*hi
    add(ones_n, s2[0])
    # mid-level terms
    add(s1[1], ones_m)
    add(ones_n, s2[1])
    for d in range(3):
        add(a1[0][:, d], n2[1][:, d])  # hi*mid
    for d in range(3):
        add(a1[1][:, d], n2[0][:, d])  # mid*hi
    for d in range(3):
        add(a1[1][:, d], n2[1][:, d])  # mid*mid
    # low-level terms
    add(s1[2], ones_m)
    add(ones_n, s2[2])
    for d in range(3):
        add(a1[0][:, d], n2[2][:, d])  # hi*lo
    for d in range(3):
        add(a1[2][:, d], n2[0][:, d])  # lo*hi
    for d in range(3):
        add(a1[1][:, d], n2[2][:, d])  # mid*lo
    for d in range(3):
        add(a1[2][:, d], n2[1][:, d])  # lo*mid
    for d in range(3):
        add(a1[2][:, d], n2[2][:, d])  # lo*lo

    lhs = np.ascontiguousarray(np.stack(lhs_rows))
    rhs = np.ascontiguousarray(np.stack(rhs_rows))
    assert lhs.shape == (K_AUG, NC_N) and rhs.shape == (K_AUG, M)
    return lhs, rhs


_CACHED_NC = None


def _get_nc():
    global _CACHED_NC
    if _CACHED_NC is None:
        _CACHED_NC = build_program()
    return _CACHED_NC


def kernel(xyz1, xyz2, _return_timing=False, _trace=False):
    xyz1 = np.asarray(xyz1, dtype=np.float32)
    xyz2 = np.asarray(xyz2, dtype=np.float32)
    assert xyz1.shape == (B, N, 3) and xyz2.shape == (B, M, 3)

    ident = np.eye(128, dtype=np.float16)
    in_maps = []
    for c in range(N_CORES):
        b, h = divmod(c, 2)
        lhs, rhs = _make_core_inputs(xyz1[b, h * NC_N : (h + 1) * NC_N], xyz2[b])
        in_maps.append({"lhs": lhs, "rhs": rhs, "ident": ident})

    nc = _get_nc()
    res = run_bass_kernel_spmd(
        nc, in_maps, core_ids=list(range(N_CORES)), trace=_trace
    )

    total = 0.0
    for b in range(B):
        row_parts = []
        col_parts = []
        for h in range(2):
            r = res.results[2 * b + h]
            row_parts.append(
                np.asarray(r["rowmin"]).astype(np.float64).T.reshape(-1)
            )  # (4096,)
            col_parts.append(
                np.asarray(r["colmin"]).astype(np.float64).T.reshape(-1)
            )  # (8192,)
        min1_d2 = np.concatenate(row_parts) / D2_SCALE  # (8192,)
        min2_d2 = np.minimum(col_parts[0], col_parts[1]) / D2_SCALE  # (8192,)
        min1 = np.sqrt(np.maximum(min1_d2, 0.0))
        min2 = np.sqrt(np.maximum(min2_d2, 0.0))
        total += min1.mean() + min2.mean()
    out = np.asarray(total / B, dtype=np.float32)
    if _return_timing:
        return out, res
    return out
